# revision 16
# baseline (speedup 1.0000x reference)
"""Trainium2 Bass kernel for nn_CrossAttention_gate (sparse cross-attention, B=1).

Strategy (8 NeuronCores, sequence-parallel over the 4096 query pixels):
  - Each core owns NL=512 query pixels. Projections (q,k,vT) are computed from
    the core's local input slice only; k/vT shards are exchanged with one
    AllGather per branch (packed [k(64x512) ; vT(512x512)] block per rank).
  - attn is built transposed [keys(part) x local-q(free)] so softmax
    normalization folds into per-partition scales and AV needs no transposes.
  - Global threshold (max/min + mean of the full attn matrix) and the global
    LayerNorm mean/var each use one tiny AllGather of per-core partials.
  - Softmax: thr mask via cmp (gpsimd) * exp (scalar engine), row sums via
    M=1 ones-matmuls; normalization fused into the AV PSUM eviction.
    All-masked rows (reference gives uniform weights) handled via
    copy_predicated with the per-key mean of v.
  - Biases folded on host: q/k biases via an augmented ones-row matmul,
    v bias exactly into b1' = b1 + W1 @ bv (softmax rows sum to 1).

Self-contained: hardcodes all shapes; no file I/O.
"""

import numpy as np

import concourse.bacc as bacc
import concourse.bass as bass
import concourse.bass_isa as bass_isa
import concourse.mybir as mybir
import concourse.tile as tile
from concourse.bass_utils import run_bass_kernel_spmd

NCORES = 8
C = 512
H = W = 64
N = H * W              # 4096 pixels
NL = N // NCORES       # 512 local query pixels
D = 64                 # head dim
KCH = N // 128         # 32 key chunks of 128
PACK = D + C           # 576 rows in the allgather pack
FP = mybir.dt.float32
FPR = mybir.dt.float32r
RG = [list(range(NCORES))]
EPS = 1e-5

# fp32r (TF32-ish PE mode, 4x faster than fp32) for matmuls whose ~5e-4
# relative error cannot flip the sparse-attention threshold mask:
# row-sums / mean-v reductions over positive values.
USE_F32R_ROWSUM = False
# fp32r for the AV product, v projection and MLP (not q/k/QK, whose values
# feed the threshold comparison and must stay bit-close to fp32).
USE_F32R_AV = False
USE_F32R_VPROJ = False
USE_F32R_MLP = False


def _mm(nc, out, lhsT, rhs, start, stop, f32r=False):
    if f32r:
        lhsT = lhsT.bitcast(FPR)
        rhs = rhs.bitcast(FPR)
    nc.tensor.matmul(out, lhsT, rhs, start=start, stop=stop)


def _emit_branch(nc, tc, ctx, P, br, qin, kv, use_max, gate_bc, xy, gam_d, bet_d,
                 out_d, ag_in, ag_out, ag_s_in, ag_s_out, ag_ln_in, ag_ln_out):
    """Emit one attention branch. br: dict of shared tiles/pools."""
    A = mybir.AluOpType
    AF = mybir.ActivationFunctionType
    X = mybir.AxisListType.X

    wq_sb, wq_b = (br["wq_sb"], br["wq_b"]) if use_max else (br["wq_sb"], br["wq_b"])
    # --- projections -------------------------------------------------------
    sp = br["sp"]   # small pool (persistent little tiles)
    ps_a = P["ps_a"]

    # q = 0.125*(Wq @ qin + bq)  -> [D, NL]
    ps_q = ps_a.tile([D, NL], FP, tag="ps_a", name=f"psq{use_max}")
    for ci in range(4):
        _mm(nc, ps_q[:], br["wq_sb"][ci][:], qin[ci][:], start=(ci == 0), stop=False)
    _mm(nc, ps_q[:], br["wq_b"][0:1, :], br["ones_row"][0:1, :], start=False, stop=True)
    q_sb = sp.tile([D, NL], FP, name=f"q{use_max}")
    nc.scalar.copy(q_sb[:], ps_q[:])

    # k (local keys) = Wk @ kv + bk -> [D, NL] -> pack rows 0:64
    ps_k = ps_a.tile([D, NL], FP, tag="ps_a", name=f"psk{use_max}")
    for ci in range(4):
        _mm(nc, ps_k[:], br["wk_sb"][ci][:], kv[ci][:], start=(ci == 0), stop=False)
    _mm(nc, ps_k[:], br["wk_b"][0:1, :], br["ones_row"][0:1, :], start=False, stop=True)
    k_sb = sp.tile([D, NL], FP, name=f"k{use_max}", tag="ksb")
    nc.scalar.copy(k_sb[:], ps_k[:])
    nc.sync.dma_start(ag_in[0:D, :], k_sb[:])

    # vT (local keys) [NL x C] (no bias; folded into b1'), pack rows 64:576
    for ks in range(4):
        ps_v = ps_a.tile([128, C], FP, tag="ps_a", name=f"psv{use_max}_{ks}")
        for ci in range(4):
            _mm(nc, ps_v[:], kv[ci][:, ks * 128:(ks + 1) * 128], br["wv_sb"][ci][:],
                start=(ci == 0), stop=(ci == 3), f32r=USE_F32R_VPROJ)
        vt_s = P["m"].tile([128, NL], FP, tag="m", name=f"vts{use_max}_{ks}")
        nc.scalar.copy(vt_s[:], ps_v[:])
        nc.sync.dma_start(ag_in[D + ks * 128: D + (ks + 1) * 128, :], vt_s[:])

    # data allgather: [576, 512] per rank -> [8*576, 512]
    nc.gpsimd.collective_compute(
        "AllGather", A.bypass, replica_groups=RG,
        ins=[ag_in.opt()], outs=[ag_out.opt()],
    )
    return q_sb


def _emit_branch_attn(nc, tc, ctx, P, br, q_sb, use_max, gate_bc, xy, gam_d, bet_d,
                      out_d, ag_out, ag_s_in, ag_s_out, ag_ln_in, ag_ln_out,
                      apply_stage):
    A = mybir.AluOpType
    AF = mybir.ActivationFunctionType
    X = mybir.AxisListType.X
    sp = br["sp"]
    tagb = "1" if use_max else "2"

    # --- QK + raw stats ----------------------------------------------------
    k_full = sp.tile([D, N], FP, name=f"kfull{tagb}", tag="kfull", bufs=1)
    for r in range(NCORES):
        nc.sync.dma_start(k_full[:, r * NL:(r + 1) * NL],
                          ag_out[r * PACK: r * PACK + D, :])

    sumbuf = sp.tile([128, KCH], FP, name=f"sumbuf{tagb}", tag="sumbuf")
    extbuf = sp.tile([128, KCH], FP, name=f"extbuf{tagb}", tag="extbuf")
    attn = []
    for kc in range(KCH):
        ps = P["ps_a"].tile([128, NL], FP, tag="ps_a", name=f"psqk{tagb}_{kc}")
        _mm(nc, ps[:], k_full[:, kc * 128:(kc + 1) * 128], q_sb[:],
            start=True, stop=True)
        t = P["attn"].tile([128, NL], FP, tag="attn", name=f"attn{tagb}_{kc}")
        nc.scalar.activation(t[:], ps[:], AF.Copy,
                             accum_out=sumbuf[:, kc:kc + 1])
        nc.vector.tensor_reduce(extbuf[:, kc:kc + 1], ps[:], axis=X,
                                op=(A.max if use_max else A.min))
        attn.append(t)

    # --- local -> global stats (one tiny allgather) ------------------------
    sumloc = sp.tile([128, 1], FP, name=f"sumloc{tagb}")
    extloc = sp.tile([128, 1], FP, name=f"extloc{tagb}")
    nc.vector.tensor_reduce(sumloc[:], sumbuf[:], axis=X, op=A.add)
    nc.vector.tensor_reduce(extloc[:], extbuf[:], axis=X,
                            op=(A.max if use_max else A.min))
    if not use_max:
        nc.scalar.mul(extloc[:], extloc[:], -1.0)  # min via max(-x)
    sumred = sp.tile([128, 1], FP, name=f"sumred{tagb}")
    extred = sp.tile([128, 1], FP, name=f"extred{tagb}")
    nc.gpsimd.partition_all_reduce(sumred[:], sumloc[:], channels=128,
                                   reduce_op=bass_isa.ReduceOp.add)
    nc.gpsimd.partition_all_reduce(extred[:], extloc[:], channels=128,
                                   reduce_op=bass_isa.ReduceOp.max)
    st_sb = sp.tile([1, 2], FP, name=f"stsb{tagb}")
    nc.vector.tensor_copy(st_sb[0:1, 0:1], extred[0:1, :])
    nc.vector.tensor_copy(st_sb[0:1, 1:2], sumred[0:1, :])
    nc.sync.dma_start(ag_s_in[:, :], st_sb[:])
    nc.gpsimd.collective_compute("AllGather", A.bypass, replica_groups=RG,
                                 ins=[ag_s_in.opt()], outs=[ag_s_out.opt()])
    st_all = sp.tile([NCORES, 2], FP, name=f"stall{tagb}")
    nc.sync.dma_start(st_all[:, :], ag_s_out[:, :])
    g_ext = sp.tile([NCORES, 1], FP, name=f"gext{tagb}")
    g_sum = sp.tile([NCORES, 1], FP, name=f"gsum{tagb}")
    nc.gpsimd.partition_all_reduce(g_ext[:], st_all[:, 0:1], channels=NCORES,
                                   reduce_op=bass_isa.ReduceOp.max)
    nc.gpsimd.partition_all_reduce(g_sum[:], st_all[:, 1:2], channels=NCORES,
                                   reduce_op=bass_isa.ReduceOp.add)
    # thr = 0.5*(stat + mean);  stat = g_ext (max) or -g_ext (min branch)
    t_mean = sp.tile([1, 1], FP, name=f"tmean{tagb}")
    t_stat = sp.tile([1, 1], FP, name=f"tstat{tagb}")
    thr11 = sp.tile([1, 1], FP, name=f"thr11{tagb}")
    nc.scalar.mul(t_mean[:], g_sum[0:1, :], 0.5 / float(N) / float(N))
    nc.scalar.mul(t_stat[:], g_ext[0:1, :], 0.5 if use_max else -0.5)
    nc.vector.tensor_tensor(thr11[:], t_stat[:], t_mean[:], A.add)
    thr_v = sp.tile([128, 1], FP, name=f"thrv{tagb}")
    nc.gpsimd.partition_broadcast(thr_v[:], thr11[:], channels=128)

    # --- mask + exp (in place) --------------------------------------------
    for kc in range(KCH):
        m = P["m"].tile([128, NL], FP, tag="m", name=f"m{tagb}_{kc}")
        nc.gpsimd.tensor_scalar(m[:], attn[kc][:], thr_v[:], None, op0=A.is_ge)
        nc.scalar.activation(attn[kc][:], attn[kc][:], AF.Exp)
        nc.vector.scalar_tensor_tensor(attn[kc][:], attn[kc][:], 1.0, m[:],
                                       op0=A.bypass, op1=A.mult)

    # --- AV + row sums + mean(v) ------------------------------------------
    ps_av = [P["ps_av"].tile([128, NL], FP, tag="ps_av", name=f"psav{tagb}_{cs}")
             for cs in range(4)]
    ps_rs = P["ps_row"].tile([1, NL], FP, tag="ps_row", name=f"psrs{tagb}")
    ps_mv = P["ps_row"].tile([1, C], FP, tag="ps_row", name=f"psmv{tagb}")
    for kc in range(KCH):
        r, s = kc // 4, kc % 4
        vtt = P["vt"].tile([128, C], FP, tag="vt", name=f"vt{tagb}_{kc}")
        nc.sync.dma_start(
            vtt[:], ag_out[r * PACK + D + s * 128: r * PACK + D + (s + 1) * 128, :])
        _mm(nc, ps_rs[:], br["ones_col"][:], attn[kc][:],
            start=(kc == 0), stop=(kc == KCH - 1), f32r=USE_F32R_ROWSUM)
        for cs in range(4):
            _mm(nc, ps_av[cs][:], vtt[:, cs * 128:(cs + 1) * 128], attn[kc][:],
                start=(kc == 0), stop=(kc == KCH - 1), f32r=USE_F32R_AV)
        _mm(nc, ps_mv[:], br["ones_col"][:], vtt[:],
            start=(kc == 0), stop=(kc == KCH - 1), f32r=USE_F32R_ROWSUM)

    rs_r = sp.tile([1, NL], FP, name=f"rsr{tagb}")
    z_r = sp.tile([1, NL], FP, name=f"zr{tagb}")
    nc.vector.reciprocal(rs_r[:], ps_rs[:])
    nc.vector.tensor_scalar(z_r[:], ps_rs[:], 0.0, None, op0=A.is_equal)
    rb = sp.tile([128, NL], FP, name=f"rb{tagb}", tag="rb")
    zb = sp.tile([128, NL], FP, name=f"zb{tagb}", tag="zb")
    nc.gpsimd.partition_broadcast(rb[:], rs_r[:], channels=128)
    nc.gpsimd.partition_broadcast(zb[:], z_r[:], channels=128)
    mv_row = sp.tile([1, C], FP, name=f"mvrow{tagb}")
    nc.scalar.mul(mv_row[:], ps_mv[:], 1.0 / float(N))
    av = []
    for cs in range(4):
        mv_col = sp.tile([128, 1], FP, name=f"mvcol{tagb}_{cs}")
        nc.sync.dma_start(mv_col[:, 0:1], mv_row[0:1, cs * 128:(cs + 1) * 128])
        mvb = P["m"].tile([128, NL], FP, tag="m", name=f"mvb{tagb}_{cs}")
        nc.scalar.activation(mvb[:], br["ones_tile"][:], AF.Copy,
                             scale=mv_col[:, 0:1])
        a = P["av"].tile([128, NL], FP, tag="av", name=f"av{tagb}_{cs}")
        nc.vector.scalar_tensor_tensor(a[:], ps_av[cs][:], 1.0, rb[:],
                                       op0=A.bypass, op1=A.mult)
        nc.vector.copy_predicated(a[:], zb[:].bitcast(mybir.dt.uint32), mvb[:])
        av.append(a)

    # --- MLP (W1/W2 streamed per branch to save SBUF) ----------------------
    w1s = []
    for ci in range(4):
        t = P["ws"].tile([128, C], FP, tag="ws", name=f"w1s{tagb}_{ci}")
        nc.sync.dma_start(t[:], br["w1_d"][ci * 128:(ci + 1) * 128, :])
        w1s.append(t)
    h1 = []
    for os_ in range(4):
        ps_h = P["ps_a"].tile([128, NL], FP, tag="ps_a", name=f"psh1{tagb}_{os_}")
        for ci in range(4):
            _mm(nc, ps_h[:], w1s[ci][:, os_ * 128:(os_ + 1) * 128],
                av[ci][:], start=(ci == 0), stop=(ci == 3), f32r=USE_F32R_MLP)
        h = P["h1"].tile([128, NL], FP, tag="h1", name=f"h1{tagb}_{os_}")
        nc.scalar.activation(h[:], ps_h[:], AF.Relu,
                             bias=br["b1_sb"][:, os_:os_ + 1], scale=1.0)
        h1.append(h)
    lnsum = sp.tile([128, 4], FP, name=f"lnsum{tagb}")
    lnsq = sp.tile([128, 4], FP, name=f"lnsq{tagb}")
    w2s = []
    for ci in range(4):
        t = P["ws"].tile([128, C], FP, tag="ws", name=f"w2s{tagb}_{ci}")
        nc.sync.dma_start(t[:], br["w2_d"][ci * 128:(ci + 1) * 128, :])
        w2s.append(t)
    h2 = []
    for os_ in range(4):
        ps_h = P["ps_a"].tile([128, NL], FP, tag="ps_a", name=f"psh2{tagb}_{os_}")
        for ci in range(4):
            _mm(nc, ps_h[:], w2s[ci][:, os_ * 128:(os_ + 1) * 128],
                h1[ci][:], start=(ci == 0), stop=(ci == 3), f32r=USE_F32R_MLP)
        h = P["h2"].tile([128, NL], FP, tag="h2", name=f"h2{tagb}_{os_}")
        nc.scalar.activation(h[:], ps_h[:], AF.Identity,
                             bias=br["b2_sb"][:, os_:os_ + 1], scale=1.0,
                             accum_out=lnsum[:, os_:os_ + 1])
        sq = P["m"].tile([128, NL], FP, tag="m", name=f"sq{tagb}_{os_}")
        nc.scalar.activation(sq[:], h[:], AF.Square,
                             accum_out=lnsq[:, os_:os_ + 1])
        h2.append(h)

    # --- LayerNorm partials + allgather ------------------------------------
    lnS = sp.tile([128, 1], FP, name=f"lnS{tagb}")
    lnQ = sp.tile([128, 1], FP, name=f"lnQ{tagb}")
    nc.vector.tensor_reduce(lnS[:], lnsum[:], axis=X, op=A.add)
    nc.vector.tensor_reduce(lnQ[:], lnsq[:], axis=X, op=A.add)
    Sred = sp.tile([128, 1], FP, name=f"Sred{tagb}")
    Qred = sp.tile([128, 1], FP, name=f"Qred{tagb}")
    nc.gpsimd.partition_all_reduce(Sred[:], lnS[:], channels=128,
                                   reduce_op=bass_isa.ReduceOp.add)
    nc.gpsimd.partition_all_reduce(Qred[:], lnQ[:], channels=128,
                                   reduce_op=bass_isa.ReduceOp.add)
    ln_sb = sp.tile([1, 2], FP, name=f"lnsb{tagb}")
    nc.vector.tensor_copy(ln_sb[0:1, 0:1], Sred[0:1, :])
    nc.vector.tensor_copy(ln_sb[0:1, 1:2], Qred[0:1, :])
    nc.sync.dma_start(ag_ln_in[:, :], ln_sb[:])
    nc.gpsimd.collective_compute("AllGather", A.bypass, replica_groups=RG,
                                 ins=[ag_ln_in.opt()], outs=[ag_ln_out.opt()])

    def apply():
        ln_all = sp.tile([NCORES, 2], FP, name=f"lnall{tagb}")
        nc.sync.dma_start(ln_all[:, :], ag_ln_out[:, :])
        S8 = sp.tile([NCORES, 1], FP, name=f"S8{tagb}")
        Q8 = sp.tile([NCORES, 1], FP, name=f"Q8{tagb}")
        nc.gpsimd.partition_all_reduce(S8[:], ln_all[:, 0:1], channels=NCORES,
                                       reduce_op=bass_isa.ReduceOp.add)
        nc.gpsimd.partition_all_reduce(Q8[:], ln_all[:, 1:2], channels=NCORES,
                                       reduce_op=bass_isa.ReduceOp.add)
        cnt = float(C) * float(N)
        mu = sp.tile([1, 1], FP, name=f"mu{tagb}")
        msq = sp.tile([1, 1], FP, name=f"msq{tagb}")
        nc.scalar.mul(mu[:], S8[0:1, :], 1.0 / cnt)
        nc.scalar.mul(msq[:], Q8[0:1, :], 1.0 / cnt)
        mu2 = sp.tile([1, 1], FP, name=f"mu2{tagb}")
        var = sp.tile([1, 1], FP, name=f"var{tagb}")
        nc.vector.tensor_tensor(mu2[:], mu[:], mu[:], A.mult)
        nc.vector.tensor_tensor(var[:], msq[:], mu2[:], A.subtract)
        sd = sp.tile([1, 1], FP, name=f"sd{tagb}")
        nc.scalar.activation(sd[:], var[:], AF.Sqrt,
                             bias=br["eps11"][0:1, 0:1], scale=1.0)
        rstd = sp.tile([1, 1], FP, name=f"rstd{tagb}")
        nc.vector.reciprocal(rstd[:], sd[:])
        nmr = sp.tile([1, 1], FP, name=f"nmr{tagb}")
        nc.vector.tensor_tensor(nmr[:], mu[:], rstd[:], A.mult)
        nc.scalar.mul(nmr[:], nmr[:], -1.0)
        rstd_v = sp.tile([128, 1], FP, name=f"rstdv{tagb}")
        nmr_v = sp.tile([128, 1], FP, name=f"nmrv{tagb}")
        nc.gpsimd.partition_broadcast(rstd_v[:], rstd[:], channels=128)
        nc.gpsimd.partition_broadcast(nmr_v[:], nmr[:], channels=128)
        for cs in range(4):
            nc.scalar.activation(h2[cs][:], h2[cs][:], AF.Identity,
                                 bias=nmr_v[:, 0:1], scale=rstd_v[:, 0:1])
            g_t = P["gb"].tile([128, NL], FP, tag="gb", name=f"g{tagb}_{cs}")
            b_t = P["gb"].tile([128, NL], FP, tag="gb", name=f"b{tagb}_{cs}")
            nc.sync.dma_start(g_t[:], gam_d[cs * 128:(cs + 1) * 128, :])
            nc.sync.dma_start(b_t[:], bet_d[cs * 128:(cs + 1) * 128, :])
            nc.vector.tensor_tensor(h2[cs][:], h2[cs][:], g_t[:], A.mult)
            nc.vector.tensor_tensor(h2[cs][:], h2[cs][:], b_t[:], A.add)
            # gate: out = (h - x)*g + x
            d = P["m"].tile([128, NL], FP, tag="m", name=f"d{tagb}_{cs}")
            nc.vector.tensor_tensor(d[:], h2[cs][:], xy[cs][:], A.subtract)
            nc.vector.tensor_tensor(d[:], d[:], gate_bc[:], A.mult)
            o = P["m"].tile([128, NL], FP, tag="m", name=f"o{tagb}_{cs}")
            nc.vector.tensor_tensor(o[:], d[:], xy[cs][:], A.add)
            nc.sync.dma_start(out_d[cs * 128:(cs + 1) * 128, :], o[:])

    apply_stage.append(apply)


def _build():
    A = mybir.AluOpType
    nc = bacc.Bacc("TRN2", target_bir_lowering=False, debug=False,
                   num_devices=NCORES)

    def din(name, shape):
        return nc.dram_tensor(name, shape, FP, kind="ExternalInput").ap()

    def dout(name, shape):
        return nc.dram_tensor(name, shape, FP, kind="ExternalOutput").ap()

    x_d = din("x_loc", [C, NL])
    y_d = din("y_loc", [C, NL])
    gam_d = din("gam_loc", [C, NL])
    bet_d = din("bet_loc", [C, NL])
    srow_d = din("s_row", [1, NL])
    qmrow_d = din("qm_row", [1, NL])
    wq_d = din("wq_pack", [C + 1, D])
    wk_d = din("wk_pack", [C + 1, D])
    wv_d = din("wv_t", [C, C])
    w1_d = din("w1_t", [C, C])
    w2_d = din("w2_t", [C, C])
    b1_d = din("b1_col", [C, 1])
    b2_d = din("b2_col", [C, 1])
    outx_d = dout("outx_loc", [C, NL])
    outy_d = dout("outy_loc", [C, NL])

    from contextlib import ExitStack
    with tile.TileContext(nc) as tc, ExitStack() as ctx:
        dram = ctx.enter_context(tc.tile_pool(name="dram", bufs=1, space="DRAM"))
        ag_in = [dram.tile([PACK, NL], FP, name=f"agin{b}") for b in range(2)]
        ag_out = [dram.tile([NCORES * PACK, NL], FP, name=f"agout{b}",
                            addr_space="Shared") for b in range(2)]
        ag_s_in = [dram.tile([1, 2], FP, name=f"agsin{b}") for b in range(2)]
        ag_s_out = [dram.tile([NCORES, 2], FP, name=f"agsout{b}",
                              addr_space="Shared") for b in range(2)]
        ag_ln_in = [dram.tile([1, 2], FP, name=f"aglnin{b}") for b in range(2)]
        ag_ln_out = [dram.tile([NCORES, 2], FP, name=f"aglnout{b}",
                               addr_space="Shared") for b in range(2)]

        P = {}
        P["attn"] = ctx.enter_context(tc.tile_pool(name="attn", bufs=32))
        P["vt"] = ctx.enter_context(tc.tile_pool(name="vt", bufs=3))
        P["m"] = ctx.enter_context(tc.tile_pool(name="m", bufs=3))
        P["av"] = ctx.enter_context(tc.tile_pool(name="av", bufs=4))
        P["h1"] = ctx.enter_context(tc.tile_pool(name="h1", bufs=4))
        P["h2"] = ctx.enter_context(tc.tile_pool(name="h2", bufs=8))
        P["gb"] = ctx.enter_context(tc.tile_pool(name="gb", bufs=2))
        P["ws"] = ctx.enter_context(tc.tile_pool(name="ws", bufs=4))
        P["ps_a"] = ctx.enter_context(tc.tile_pool(name="ps_a", bufs=2, space="PSUM"))
        P["ps_av"] = ctx.enter_context(tc.tile_pool(name="ps_av", bufs=4, space="PSUM"))
        P["ps_row"] = ctx.enter_context(tc.tile_pool(name="ps_row", bufs=2, space="PSUM"))

        cp = ctx.enter_context(tc.tile_pool(name="const", bufs=1))
        br = {"sp": cp}

        # constants & weights
        ones_row = cp.tile([1, NL], FP, name="ones_row")
        nc.vector.memset(ones_row[:], 1.0)
        ones_col = cp.tile([128, 1], FP, name="ones_col")
        nc.vector.memset(ones_col[:], 1.0)
        ones_tile = cp.tile([128, NL], FP, name="ones_tile")
        nc.vector.memset(ones_tile[:], 1.0)
        eps11 = cp.tile([1, 1], FP, name="eps11")
        nc.vector.memset(eps11[:], EPS)
        br.update(ones_row=ones_row, ones_col=ones_col, ones_tile=ones_tile,
                  eps11=eps11)

        def load4(name, src, shape, colslice=None):
            ts = []
            for i in range(4):
                t = cp.tile(shape, FP, name=f"{name}{i}")
                s = src[i * 128:(i + 1) * 128, :] if colslice is None else src
                nc.sync.dma_start(t[:], s)
                ts.append(t)
            return ts

        br["wq_sb"] = load4("wq", wq_d, [128, D])
        br["wk_sb"] = load4("wk", wk_d, [128, D])
        wq_b = cp.tile([1, D], FP, name="wqb")
        nc.sync.dma_start(wq_b[:], wq_d[C:C + 1, :])
        wk_b = cp.tile([1, D], FP, name="wkb")
        nc.sync.dma_start(wk_b[:], wk_d[C:C + 1, :])
        br["wq_b"], br["wk_b"] = wq_b, wk_b
        br["wv_sb"] = load4("wv", wv_d, [128, C])
        br["w1_d"], br["w2_d"] = w1_d, w2_d
        b1_sb = cp.tile([128, 4], FP, name="b1sb")
        b2_sb = cp.tile([128, 4], FP, name="b2sb")
        for i in range(4):
            nc.sync.dma_start(b1_sb[:, i:i + 1], b1_d[i * 128:(i + 1) * 128, :])
            nc.sync.dma_start(b2_sb[:, i:i + 1], b2_d[i * 128:(i + 1) * 128, :])
        br["b1_sb"], br["b2_sb"] = b1_sb, b2_sb

        # inputs + gate masks
        x_sb = load4("xsb", x_d, [128, NL])
        y_sb = load4("ysb", y_d, [128, NL])
        srow = cp.tile([1, NL], FP, name="srow")
        qmrow = cp.tile([1, NL], FP, name="qmrow")
        nc.sync.dma_start(srow[:], srow_d[:, :])
        nc.sync.dma_start(qmrow[:], qmrow_d[:, :])
        s_bc = cp.tile([128, NL], FP, name="s_bc")
        qm_bc = cp.tile([128, NL], FP, name="qm_bc")
        nc.gpsimd.partition_broadcast(s_bc[:], srow[:], channels=128)
        nc.gpsimd.partition_broadcast(qm_bc[:], qmrow[:], channels=128)

        # masked inputs
        fg = ctx.enter_context(tc.tile_pool(name="fg", bufs=4))
        x_fg, y_fg = [], []
        for i in range(4):
            t = fg.tile([128, NL], FP, tag="fg", name=f"xfg{i}")
            nc.vector.tensor_tensor(t[:], x_sb[i][:], s_bc[:], A.mult)
            x_fg.append(t)
        for i in range(4):
            t = fg.tile([128, NL], FP, tag="fg", name=f"yfg{i}")
            nc.vector.tensor_tensor(t[:], y_sb[i][:], qm_bc[:], A.mult)
            y_fg.append(t)

        # branch 1: q from x_fg, k/v from y_fg, max-threshold, gate s
        q1 = _emit_branch(nc, tc, ctx, P, br, x_fg, y_fg, True, s_bc, x_sb,
                          gam_d, bet_d, outx_d, ag_in[0], ag_out[0],
                          ag_s_in[0], ag_s_out[0], ag_ln_in[0], ag_ln_out[0])
        # branch 2: q from y_fg, k/v from raw x, min-threshold, gate q_mb
        q2 = _emit_branch(nc, tc, ctx, P, br, y_fg, x_sb, False, qm_bc, y_sb,
                          gam_d, bet_d, outy_d, ag_in[1], ag_out[1],
                          ag_s_in[1], ag_s_out[1], ag_ln_in[1], ag_ln_out[1])

        apply_stage = []
        _emit_branch_attn(nc, tc, ctx, P, br, q1, True, s_bc, x_sb, gam_d, bet_d,
                          outx_d, ag_out[0], ag_s_in[0], ag_s_out[0],
                          ag_ln_in[0], ag_ln_out[0], apply_stage)
        _emit_branch_attn(nc, tc, ctx, P, br, q2, False, qm_bc, y_sb, gam_d,
                          bet_d, outy_d, ag_out[1], ag_s_in[1], ag_s_out[1],
                          ag_ln_in[1], ag_ln_out[1], apply_stage)
        for f in apply_stage:
            f()

    nc.compile()
    return nc


_CACHED_NC = None


def _get_nc():
    global _CACHED_NC
    if _CACHED_NC is None:
        _CACHED_NC = _build()
    return _CACHED_NC


def _host_pack(inputs):
    f = np.float32
    x = np.ascontiguousarray(inputs["x"].reshape(C, N), dtype=f)
    y = np.ascontiguousarray(inputs["y"].reshape(C, N), dtype=f)
    gam = np.ascontiguousarray(inputs["gamma"].reshape(C, N), dtype=f)
    bet = np.ascontiguousarray(inputs["beta"].reshape(C, N), dtype=f)
    s_f = inputs["s_m"].reshape(N).astype(f)
    qm_f = (inputs["q_m"].reshape(N) > 0.8).astype(f)
    Wq, bq = inputs["Wq"].astype(f), inputs["bq"].astype(f)
    Wk, bk = inputs["Wk"].astype(f), inputs["bk"].astype(f)
    Wv, bv = inputs["Wv"].astype(f), inputs["bv"].astype(f)
    W1, b1 = inputs["W1"].astype(f), inputs["b1"].astype(f)
    W2, b2 = inputs["W2"].astype(f), inputs["b2"].astype(f)
    scale = (C // 8) ** (-0.5)

    wq_pack = np.concatenate([(Wq * scale).T, (bq * scale)[None, :]], 0)
    wk_pack = np.concatenate([Wk.T, bk[None, :]], 0)
    b1p = b1 + W1 @ bv

    shared = {
        "wq_pack": np.ascontiguousarray(wq_pack, dtype=f),
        "wk_pack": np.ascontiguousarray(wk_pack, dtype=f),
        "wv_t": np.ascontiguousarray(Wv.T, dtype=f),
        "w1_t": np.ascontiguousarray(W1.T, dtype=f),
        "w2_t": np.ascontiguousarray(W2.T, dtype=f),
        "b1_col": np.ascontiguousarray(b1p[:, None], dtype=f),
        "b2_col": np.ascontiguousarray(b2[:, None], dtype=f),
    }
    in_maps = []
    for r in range(NCORES):
        sl = slice(r * NL, (r + 1) * NL)
        m = dict(shared)
        m["x_loc"] = np.ascontiguousarray(x[:, sl])
        m["y_loc"] = np.ascontiguousarray(y[:, sl])
        m["gam_loc"] = np.ascontiguousarray(gam[:, sl])
        m["bet_loc"] = np.ascontiguousarray(bet[:, sl])
        m["s_row"] = np.ascontiguousarray(s_f[None, sl])
        m["qm_row"] = np.ascontiguousarray(qm_f[None, sl])
        in_maps.append(m)
    return in_maps


def _gather(results):
    outx = np.empty((C, N), np.float32)
    outy = np.empty((C, N), np.float32)
    for r in range(NCORES):
        sl = slice(r * NL, (r + 1) * NL)
        outx[:, sl] = results[r]["outx_loc"]
        outy[:, sl] = results[r]["outy_loc"]
    return outx.reshape(1, C, H, W), outy.reshape(1, C, H, W)


def kernel(**inputs):
    nc = _get_nc()
    in_maps = _host_pack(inputs)
    res = run_bass_kernel_spmd(nc, in_maps, core_ids=list(range(NCORES)))
    return _gather(res.results)


if __name__ == "__main__":
    import reference
    inp = {k: np.asarray(v) for k, v in reference.setup_inputs().items()}
    ox, oy = kernel(**inp)
    ex, ey = reference.reference(**reference.setup_inputs())
    for name, a, b in [("outx", ox, np.asarray(ex)), ("outy", oy, np.asarray(ey))]:
        err = np.max(np.abs(a - b)) / max(np.max(np.abs(b)), 1e-9)
        print(name, "rel absmax err:", err)
